# revision 2
# baseline (speedup 1.0000x reference)
"""Causal multi-head self-attention on 8 trn2 NeuronCores.

Sharding: 8 cores = batch(4) x head-group(2).  Each core computes attention
for 6 of the 12 heads of one batch element, plus its partial output
projection; the host sums the two partials per batch element.

Per-core kernel (Bass/Tile).  All matmul operands are bf16 (fp32 PSUM
accumulation); inputs are converted host-side, partial outputs return as
bf16 and are summed in fp32 on the host.

  phase 1: project qT/kT [d, S] m-tiles and v [S, 65/head] (ones column
    per head yields softmax denominators later).
  phase 2: per head, per 512-wide q chunk: for each pair of 128-wide
    k blocks: scoresT[k,q] = kT.T @ qT (PSUM), pT = exp(scoresT) on
    ScalarE, causal handling restricted to the valid suffix of diagonal
    blocks (triangular [128,128] zeroed via gpsimd.affine_select), then
    oT[65,512] += v_ext.T @ pT.  Normalize attnT = oT[0:64] * 1/oT[64]
    into qT's storage.
  phase 3: out[S,768] partial = attnT.T @ woT, written as bf16.
"""

import numpy as np
import ml_dtypes

import concourse.bass as bass
import concourse.bacc as bacc
import concourse.mybir as mybir
import concourse.tile as tile
from concourse.bass_utils import run_bass_kernel_spmd

F32 = mybir.dt.float32
BF16 = mybir.dt.bfloat16

B, S, D = 4, 2048, 768
H = 12          # total heads
DK = 64         # head dim
HPC = 6         # heads per core
GC = HPC * DK   # 384 cols per head-group
P = 128
KT = D // P     # 6 k-tiles over d_model
MT = GC // P    # 3 tiles over the 384 group cols
NQ = S // 512   # 4 q chunks of 512
SBLK = S // P   # 16 sequence blocks of 128


def _attention_head(nc, ps_sc, ps_o, att_w, att_n, qa_sb, kT_sb, v_sb, h, no_mask=False):
    hp = 64 * (h % 2)
    ht = h // 2
    for j in range(NQ):
        nblk = 4 * (j + 1)  # causal: k blocks 0..nblk-1
        po = ps_o.tile([DK + 1, 512], F32, name="po", tag="po")
        for g in range(nblk // 2):
            diag = g >= 2 * j  # groups 2j, 2j+1 hold the diagonal blocks
            ps = ps_sc.tile([P, 2, 512], F32, name="ps", tag="ps")
            for i in range(2):
                b = 2 * g + i
                off = max(0, 128 * b - 512 * j) if not no_mask else 0
                nc.tensor.matmul(
                    ps[:, i, off:512],
                    kT_sb[hp : hp + DK, ht, b * P : (b + 1) * P],
                    qa_sb[hp : hp + DK, ht, j * 512 + off : (j + 1) * 512],
                    start=True,
                    stop=True,
                )
            pt = att_w.tile([P, 2, 512], BF16, name="pt", tag="pt")
            if diag and not no_mask:
                for i in range(2):
                    b = 2 * g + i
                    bi = 128 * b - 512 * j
                    nc.scalar.activation(
                        pt[:, i, bi:512], ps[:, i, bi:512],
                        mybir.ActivationFunctionType.Exp,
                    )
                    # triangular boundary block: zero pT where k > q
                    nc.gpsimd.affine_select(
                        out=pt[:, i, bi : bi + 128],
                        in_=pt[:, i, bi : bi + 128],
                        compare_op=mybir.AluOpType.is_ge,
                        fill=0.0,
                        base=0,
                        pattern=[[1, 128]],
                        channel_multiplier=-1,
                    )
            else:
                nc.scalar.activation(pt[:], ps[:], mybir.ActivationFunctionType.Exp)
            for i in range(2):
                b = 2 * g + i
                off = max(0, 128 * b - 512 * j) if not no_mask else 0
                nc.tensor.matmul(
                    po[:, off:512],
                    v_sb[:, b, h, :],
                    pt[:, i, off:512],
                    start=(b == 0),
                    stop=(b == nblk - 1),
                )
        # normalize: attnT = po[0:64] / po[64], written into qT's storage.
        # NB: partition_broadcast reads PHYSICAL partition 0 on HW (ignores
        # the AP base), so land the reciprocal at base 0 first (DVE handles
        # the cross-partition-base shift).
        rec = att_n.tile([1, 512], F32, name="rec", tag="rec")
        nc.vector.reciprocal(rec[:], po[DK : DK + 1, :])
        recb = att_n.tile([DK, 512], F32, name="recb", tag="recb")
        nc.gpsimd.partition_broadcast(recb[:], rec[:])
        nc.vector.tensor_mul(
            qa_sb[hp : hp + DK, ht, j * 512 : (j + 1) * 512],
            po[0:DK, :],
            recb[:],
        )


def _emit(nc, tc, d, r, no_mask=False):
    """Emit one full forward pass. d = dict of DRAM APs, r = rep index."""
    with tc.tile_pool(name=f"persist{r}", bufs=1) as persist:
        qa_sb = persist.tile([P, MT, S], BF16, name="qa_sb")   # qT, then attnT
        kT_sb = persist.tile([P, MT, S], BF16, name="kT_sb")
        # v natural + ones column per head: [p, sblk, head, 65]
        v_sb = persist.tile([P, SBLK, HPC, DK + 1], BF16, name="v_sb")
        wo_sb = persist.tile([P, MT, D], BF16, name="wo_sb")

        # write ones via f32 memset + DVE copy (cast to bf16)
        ones_f32 = persist.tile([P, SBLK * HPC], F32, name="ones_f32")
        nc.vector.memset(ones_f32[:], 1.0)
        nc.vector.tensor_copy(
            v_sb[:, :, :, DK], ones_f32.rearrange("p (t h) -> p t h", t=SBLK)
        )
        nc.sync.dma_start(wo_sb[:], d["woT"].rearrange("(t p) n -> p t n", p=P))

        with tc.tile_pool(name=f"xw{r}", bufs=1) as xw:
            xT_sb = xw.tile([P, KT, S], BF16, name="xT_sb")
            wq_sb = xw.tile([P, KT, GC], BF16, name="wq_sb")
            wk_sb = xw.tile([P, KT, GC], BF16, name="wk_sb")
            wv_sb = xw.tile([P, KT, GC], BF16, name="wv_sb")

            nc.sync.dma_start(wq_sb[:], d["wqT"].rearrange("(t p) n -> p t n", p=P))
            xT_r = d["xT"].rearrange("(t p) s -> p t s", p=P)
            for k in range(KT):
                nc.sync.dma_start(xT_sb[:, k, :], xT_r[:, k, :])
            nc.sync.dma_start(wk_sb[:], d["wkT"].rearrange("(t p) n -> p t n", p=P))
            nc.sync.dma_start(wv_sb[:], d["wvT"].rearrange("(t p) n -> p t n", p=P))

            # ---- phase 1: projections ----
            with tc.tile_pool(name=f"ps1{r}", bufs=4, space="PSUM") as ps1:
                for w_sb, dst in ((wq_sb, qa_sb), (wk_sb, kT_sb)):
                    for m in range(MT):
                        for n in range(NQ):
                            pq = ps1.tile([P, 512], F32, name="pq", tag="pq")
                            for k in range(KT):
                                nc.tensor.matmul(
                                    pq[:],
                                    w_sb[:, k, m * P : (m + 1) * P],
                                    xT_sb[:, k, n * 512 : (n + 1) * 512],
                                    start=(k == 0),
                                    stop=(k == KT - 1),
                                )
                            nc.vector.tensor_copy(
                                dst[:, m, n * 512 : (n + 1) * 512], pq[:]
                            )
                for t in range(SBLK):
                    pv = ps1.tile([P, GC], F32, name="pv", tag="pq")
                    for k in range(KT):
                        nc.tensor.matmul(
                            pv[:],
                            xT_sb[:, k, t * P : (t + 1) * P],
                            wv_sb[:, k, :],
                            start=(k == 0),
                            stop=(k == KT - 1),
                        )
                    nc.vector.tensor_copy(
                        v_sb[:, t, :, 0:DK], pv.rearrange("p (h d) -> p h d", h=HPC)
                    )

        # ---- phase 2: attention ----
        with (
            tc.tile_pool(name=f"ps_sc{r}", bufs=2, space="PSUM") as ps_sc,
            tc.tile_pool(name=f"ps_o{r}", bufs=4, space="PSUM") as ps_o,
            tc.tile_pool(name=f"att_w{r}", bufs=3) as att_w,
            tc.tile_pool(name=f"att_n{r}", bufs=2) as att_n,
        ):
            for h in range(HPC):
                _attention_head(
                    nc, ps_sc, ps_o, att_w, att_n, qa_sb, kT_sb, v_sb, h,
                    no_mask=no_mask,
                )

        # ---- phase 3: output projection ----
        with (
            tc.tile_pool(name=f"ps3{r}", bufs=4, space="PSUM") as ps3,
            tc.tile_pool(name=f"out_w{r}", bufs=3) as out_w,
        ):
            for t in range(SBLK):
                ot = out_w.tile([P, D], BF16, name="ot", tag="ot")
                for n in range(2):
                    po3 = ps3.tile([P, 384], F32, name="po3", tag="po3")
                    for k in range(MT):
                        nc.tensor.matmul(
                            po3[:],
                            qa_sb[:, k, t * P : (t + 1) * P],
                            wo_sb[:, k, n * 384 : (n + 1) * 384],
                            start=(k == 0),
                            stop=(k == MT - 1),
                        )
                    nc.vector.tensor_copy(ot[:, n * 384 : (n + 1) * 384], po3[:])
                nc.sync.dma_start(d["out"][t * P : (t + 1) * P, :], ot[:])


def build_nc(debug_taps=False, reps=1, no_mask=False, interleave=False):
    nc = bacc.Bacc("TRN2", target_bir_lowering=False, debug=False)

    d = {
        "xT": nc.dram_tensor("xT", [D, S], BF16, kind="ExternalInput").ap(),
        "wqT": nc.dram_tensor("wqT", [D, GC], BF16, kind="ExternalInput").ap(),
        "wkT": nc.dram_tensor("wkT", [D, GC], BF16, kind="ExternalInput").ap(),
        "wvT": nc.dram_tensor("wvT", [D, GC], BF16, kind="ExternalInput").ap(),
        "woT": nc.dram_tensor("woT", [GC, D], BF16, kind="ExternalInput").ap(),
        "out": nc.dram_tensor("out", [S, D], BF16, kind="ExternalOutput").ap(),
    }

    with tile.TileContext(nc) as tc:
        for r in range(reps):
            _emit(nc, tc, d, r, no_mask=no_mask)

    nc.compile()
    return nc


_NC = None


def _get_nc():
    global _NC
    if _NC is None:
        _NC = build_nc()
    return _NC


def make_in_maps(x, wq, wk, wv, wo):
    x = np.asarray(x, np.float32)
    wq = np.asarray(wq, np.float32)
    wk = np.asarray(wk, np.float32)
    wv = np.asarray(wv, np.float32)
    wo = np.asarray(wo, np.float32)
    scale = 1.0 / np.sqrt(np.float32(DK))
    bf = ml_dtypes.bfloat16
    in_maps = []
    for c in range(8):
        b, g = divmod(c, 2)
        sl = slice(GC * g, GC * (g + 1))
        in_maps.append(
            {
                "xT": np.ascontiguousarray(x[b].T.astype(bf)),
                "wqT": np.ascontiguousarray((wq[sl, :] * scale).T.astype(bf)),
                "wkT": np.ascontiguousarray(wk[sl, :].T.astype(bf)),
                "wvT": np.ascontiguousarray(wv[sl, :].T.astype(bf)),
                "woT": np.ascontiguousarray(wo[:, sl].T.astype(bf)),
            }
        )
    return in_maps


def combine(results):
    outs = [np.asarray(r["out"], np.float32) for r in results]
    return np.stack([outs[2 * b] + outs[2 * b + 1] for b in range(B)])


def kernel(x, wq, wk, wv, wo, _trace=False):
    nc = _get_nc()
    res = run_bass_kernel_spmd(
        nc, make_in_maps(x, wq, wk, wv, wo), core_ids=list(range(8)), trace=_trace
    )
    out = combine(res.results)
    kernel.last_result = res
    return out
